# revision 38
# baseline (speedup 1.0000x reference)
"""Multi-head self-attention Trainium2 kernel (8-core head-parallel, v11).

Problem: B=2, N=2048, C=1024, H=16 heads, HD=64.  Measured NEFF exec time
~300 us per core (baseline v3: 476 us), rel err ~7e-4.

Structure (all matmuls fp16 with f32 PSUM, 2 heads per core):

  * every core receives the FULL x^T (8 MB fp16) plus its per-core qkv
    weight slices as ExternalInputs -- input shipping happens before NEFF
    execution, so there is no x AllGather, and no predicated weight-select
    DMAs in the execution window (v7: those cost a ~50 us head bubble on
    the per-queue DMA-completion counters).
  * a 64-element dummy AllGather posts at kernel start with no real
    dependencies: the ~15-60 us first-collective rendezvous barrier (NEFF
    launch skew across cores) runs concurrently with the qkv phase.
  * x streams in per 512-token block, each block split across the sync +
    scalar trigger queues; per-block SBUF tiles keep the dependencies
    block-granular so the first matmul issues at ~13 us.
  * attention runs as (batch, head, query-block) groups of 8 exp-slots
    each: score pair -> ACT exp(s/8-4) straight out of PSUM -> attn@v pair
    accumulating into PSUM [65, 512] (row 64 = denominators via a
    constant-1.0 v column; the v bias folds into the host-side output
    bias).  Scores for slot k+1 issue before attn@v of slot k so the PE
    keeps streaming while ACT runs; all remaining qkv work (blocks 1-3 of
    batch 0, all of batch 1) is sliced into single-matmul closures and
    drip-fed into the slots (due-slot scheduled just ahead of their
    consumers) so the PE never idles while ACT is the limiter.
  * normalization: PSUM denominator row -> fp32 reciprocal_approx_fast on
    DVE ([1,512]; plain DVE reciprocal is a fixed ~3.3 us regardless of
    size), partition-broadcast on the otherwise-idle GPSIMD engine,
    multiply on DVE -> oh^T fp16.
  * output exchange: one small AllToAll per (batch, head) -- chunk
    [64, 256] fp16 per peer -- posted as soon as that head's query blocks
    are done.  Each core ends up owning 256 tokens of each batch with all
    heads' features (head-halves land in partitions 0:64/64:128 so the
    local projection contracts over full 128-row chunks), and projects
    them with full fp32 PSUM accumulation.  Batch 0's AllToAll +
    projection hide under batch 1's attention; only batch 1's small
    AllToAll + 4 projection units (~25 us) are exposed in the tail.
"""

import numpy as np

B, N, C = 2, 2048, 1024
H = 16
HD = C // H  # 64
SCALE = HD ** -0.5
T = B * N  # 4096 tokens
NCORES = 8
HPC = H // NCORES  # 2 heads per core
TPC = N // NCORES  # 256 tokens per core per batch
EXP_BIAS = -4.0

_CACHE = {}


def _prep_weights(w_qkv, b_qkv, w_proj):
    """Per-core qkv weight slices (stacked, partition-major) + shared full
    w_proj in contraction-chunk-major layout."""
    CC = C // 128
    F = 3 * HPC * HD  # 384
    wq_all = np.empty((NCORES * 128, CC * F), np.float16)
    b_all = np.empty((NCORES * 128, 2), np.float32)
    for core in range(NCORES):
        heads = [core * HPC + h for h in range(HPC)]
        cols = []
        for s in range(3):  # q, k, v groups -> [qA qB kA kB vA vB]
            for h in heads:
                cols.append(np.arange(s * C + h * HD, s * C + (h + 1) * HD))
        cols = np.concatenate(cols)
        wq_core = w_qkv[:, cols].astype(np.float16)  # [C, F]
        wq_all[core * 128:(core + 1) * 128] = (
            wq_core.reshape(CC, 128, F).transpose(1, 0, 2).reshape(128, CC * F))
        b_all[core * 128:(core + 1) * 128] = (
            b_qkv[cols[:256]].reshape(2, HPC * HD).T.astype(np.float32))
    # w_proj rows grouped by contraction chunk j (= heads 2j, 2j+1):
    # w2_full[p, j*C + oc] = w_proj[j*128 + p, oc]
    w2_full = np.ascontiguousarray(
        w_proj.astype(np.float16).reshape(NCORES, 128, C)
        .transpose(1, 0, 2).reshape(128, NCORES * C))
    return wq_all, b_all, w2_full


def _build_program():
    import concourse.bass as bass
    import concourse.mybir as mybir
    import concourse.tile as tile
    from concourse import bacc

    f16 = mybir.dt.float16
    f32 = mybir.dt.float32
    Exp = mybir.ActivationFunctionType.Exp
    Mult = mybir.AluOpType.mult

    nc = bacc.Bacc("TRN2", target_bir_lowering=False, debug=False,
                   num_devices=NCORES)

    CC = C // 128          # 8 contraction chunks
    NTB = T // 512         # 8 token blocks
    NKC = N // 128         # 16 key chunks per batch
    NQB = N // 512         # 4 query blocks per batch
    NTC = T // 128         # 32 token chunks
    GROUPS = [list(range(NCORES))]

    # full x^T, token-block-major: xs[p, ((tb*CC)+ci)*512 + t]
    xs_d = nc.dram_tensor("xs", [128, NTB * CC * 512], f16,
                          kind="ExternalInput")
    out_d = nc.dram_tensor("out_sh", [B, TPC, C], f16, kind="ExternalOutput")
    import os as _os0
    dbg = bool(int(_os0.environ.get("KV4_DEBUG", "0")))
    if dbg:
        dbg_d = {nm: nc.dram_tensor(f"dbg_{nm}", shp, f16,
                                    kind="ExternalOutput")
                 for nm, shp in [("qT", [128, T]), ("kT", [128, T]),
                                 ("v", [128, 32 * 130]), ("ohT", [128, T])]}

    wq_d = nc.dram_tensor("wq", [128, CC * 3 * HPC * HD], f16,
                          kind="ExternalInput")
    b_d = nc.dram_tensor("bq", [128, 2], f32, kind="ExternalInput")
    w2_d = nc.dram_tensor("w2", [128, NCORES * C], f16,
                          kind="ExternalInput")

    dummy_in = nc.dram_tensor("dummy_in", [1, 64], f16, kind="Internal")
    dummy_out = nc.dram_tensor("dummy_out", [NCORES, 64], f16,
                               kind="Internal", addr_space="Shared")
    a2a_in = [[nc.dram_tensor(f"a2a_in{b}_{h}", [NCORES, 64, TPC], f16,
                              kind="Internal") for h in range(HPC)]
              for b in range(B)]
    a2a_out = [[nc.dram_tensor(f"a2a_out{b}_{h}", [NCORES, 64, TPC], f16,
                               kind="Internal") for h in range(HPC)]
               for b in range(B)]

    with tile.TileContext(nc) as tc:
        with tc.tile_pool(name="persist", bufs=1) as persist, \
             tc.tile_pool(name="exp", bufs=6) as exp_pool, \
             tc.tile_pool(name="small", bufs=4) as small_pool, \
             tc.tile_pool(name="gth", bufs=2) as g_pool, \
             tc.tile_pool(name="ob", bufs=4) as ob_pool, \
             tc.tile_pool(name="ps", bufs=2, space="PSUM") as psum_s, \
             tc.tile_pool(name="po", bufs=2, space="PSUM") as psum_o, \
             tc.tile_pool(name="pu", bufs=1, space="PSUM") as psum_u:

            w_sb = persist.tile([128, CC, 3 * HPC * HD], f16, tag="w_sb")
            b_sb = persist.tile([128, 2], f32, tag="b_sb")
            w2_sb = persist.tile([128, NCORES, C], f16, tag="w2_sb")
            xt_blk = [persist.tile([128, CC, 512], f16, tag=f"xt{tb}",
                                   name=f"xt{tb}")
                      for tb in range(NTB)]
            qT = persist.tile([128, T], f16, tag="qT")
            kT = persist.tile([128, T], f16, tag="kT")
            v_nat = persist.tile([128, NTC, 130], f16, tag="v_nat")
            ohT = persist.tile([128, T], f16, tag="ohT")
            bias_m4 = persist.tile([128, 1], f32, tag="bias_m4")
            dtile = persist.tile([1, 64], f16, tag="dtile")

            # ---- dummy collective first: absorbs the ~40us rendezvous
            # barrier under the qkv phase ----
            nc.vector.memset(dtile[:], 1.0)
            nc.sync.dma_start(out=dummy_in[:], in_=dtile[:])
            nc.gpsimd.collective_compute(
                "AllGather", mybir.AluOpType.bypass, replica_groups=GROUPS,
                ins=[dummy_in[:].opt()], outs=[dummy_out[:].opt()])

            # ---- weights (scalar-queue triggers: ACT is idle during the
            # head phase; keeps the sync queue clear for the x block loads,
            # and x block 0 is emitted before everything else) ----
            xs_v = xs_d[:].rearrange("p (tb cc t) -> p tb cc t", tb=NTB, cc=CC)

            def emit_x_load(tb):
                # split each block across both trigger queues so every block
                # lands at the two-queue aggregate rate, in block order
                nc.sync.dma_start(out=xt_blk[tb][:, 0:CC // 2, :],
                                  in_=xs_v[:, tb, 0:CC // 2, :])
                nc.scalar.dma_start(out=xt_blk[tb][:, CC // 2:, :],
                                    in_=xs_v[:, tb, CC // 2:, :])

            nc.scalar.dma_start(
                out=w_sb[:],
                in_=wq_d[:].rearrange("p (cc f) -> p cc f", cc=CC))
            nc.scalar.dma_start(out=b_sb[:], in_=b_d[:])
            for tb in range(NTB // 2):
                emit_x_load(tb)
            nc.sync.dma_start(
                out=w2_sb[:],
                in_=w2_d[:].rearrange("p (j c) -> p j c", j=NCORES))
            for tb in range(NTB // 2, NTB):
                emit_x_load(tb)
            nc.vector.memset(bias_m4[:], EXP_BIAS)
            nc.vector.memset(v_nat[:, :, 64:65], 1.0)
            nc.vector.memset(v_nat[:, :, 129:130], 1.0)

            # ---- qkv micro-units: lists of single-instruction closures so
            # they can be drip-fed into attention's PE stream ----
            def qk_unit_micro(tb, fc):
                st = {}

                def mk(ci):
                    def f():
                        if "ps" not in st:
                            st["ps"] = psum_u.tile(
                                [128, 512], f32, tag="pu",
                                name=f"ps1_{tb}_{fc}")
                        nc.tensor.matmul(
                            st["ps"][:],
                            w_sb[:, ci, fc * 128:(fc + 1) * 128],
                            xt_blk[tb][:, ci, :],
                            start=(ci == 0), stop=(ci == CC - 1))
                    return f

                def evac():
                    nc.vector.tensor_scalar_add(
                        (qT if fc == 0 else kT)[:, tb * 512:(tb + 1) * 512],
                        st["ps"][:], b_sb[:, fc:fc + 1])

                return [mk(ci) for ci in range(CC)] + [evac]

            def v_unit_micro(tb):
                st = {}
                cl = []

                def mkmm(tcq, ci):
                    def f():
                        if "pv" not in st:
                            st["pv"] = psum_u.tile(
                                [128, 512], f32, tag="pu", name=f"pv_{tb}")
                        nc.tensor.matmul(
                            st["pv"][:, tcq * 128:(tcq + 1) * 128],
                            xt_blk[tb][:, ci, tcq * 128:(tcq + 1) * 128],
                            w_sb[:, ci, 256:384],
                            start=(ci == 0), stop=(ci == CC - 1))
                    return f

                def mkevac(tcq):
                    def f():
                        tcg = tb * 4 + tcq
                        # strided copy: pv cols [0:64],[64:128] land at
                        # v_nat[:, tcg, 0:64] and [65:129] (skip ones col)
                        src = st["pv"][:, tcq * 128:(tcq + 1) * 128]
                        dst = v_nat[:, tcg, 0:129]
                        nc.vector.tensor_copy(
                            bass.AP(tensor=dst.tensor, offset=dst.offset,
                                    ap=[list(dst.ap[0]), [65, 2], [1, 64]]),
                            bass.AP(tensor=src.tensor, offset=src.offset,
                                    ap=[list(src.ap[0]), [64, 2], [1, 64]]))
                    return f

                for tcq in range(4):
                    cl += [mkmm(tcq, ci) for ci in range(CC)]
                    cl.append(mkevac(tcq))
                return cl

            def proj_unit_micro(b, g, tc2, jh):
                st = {}

                def mk(j):
                    def f():
                        if "pp" not in st:
                            st["pp"] = psum_u.tile(
                                [128, 512], f32, tag="pu",
                                name=f"pp_{b}_{tc2}_{jh}")
                        nc.tensor.matmul(
                            st["pp"][:],
                            g[:, j, tc2 * 128:(tc2 + 1) * 128],
                            w2_sb[:, j, jh * 512:(jh + 1) * 512],
                            start=(j == 0), stop=(j == NCORES - 1))
                    return f

                def evac():
                    ob = ob_pool.tile([128, 512], f16, tag="ob",
                                      name=f"ob_{b}_{tc2}_{jh}")
                    nc.vector.tensor_copy(ob[:], st["pp"][:])
                    nc.sync.dma_start(
                        out=out_d[b, tc2 * 128:(tc2 + 1) * 128,
                                  jh * 512:(jh + 1) * 512],
                        in_=ob[:])

                return [mk(j) for j in range(NCORES)] + [evac]

            class Feed:
                """Drip-feeds filler closures.  `sched` is a list of
                (due_slot, closure): at each tick, all closures due at or
                before the current slot are emitted (in order); the rest
                follow linear pacing over the remaining span."""

                def __init__(self, sched, span):
                    self.sched = list(sched)
                    self.span = max(1, span)
                    self.slot = 0

                def tick(self):
                    self.slot += 1
                    while self.sched and self.sched[0][0] <= self.slot:
                        self.sched.pop(0)[1]()

                def flush(self):
                    while self.sched:
                        self.sched.pop(0)[1]()

            # ---- attention group for (batch, head, query-block),
            # software-pipelined: scores(kcg+1) issue before av(kcg) so the
            # PE keeps streaming while ACT runs the exp of kcg; filler
            # closures are emitted once per kcg slot to fill the remaining
            # PE gap (keeps the p-state ramp warm) ----
            def emit_attn_group(b, h, qb, feed):
                hsl = slice(h * 64, (h + 1) * 64)
                qsl = slice(b * N + qb * 512, b * N + (qb + 1) * 512)
                po = psum_o.tile([128, 512], f32, tag="po",
                                 name=f"po_{b}_{h}_{qb}")

                def emit_scores(kcg):
                    ps = psum_s.tile([128, 1024], f32, tag="s",
                                     name=f"ps2_{b}_{h}_{qb}_{kcg}")
                    for kc2 in range(2):
                        kc = kcg * 2 + kc2
                        ksl = slice(b * N + kc * 128, b * N + (kc + 1) * 128)
                        nc.tensor.matmul(
                            ps[:, kc2 * 512:(kc2 + 1) * 512],
                            kT[hsl, ksl], qT[hsl, qsl],
                            start=True, stop=True)
                    ex = exp_pool.tile([128, 1024], f16, tag="ex",
                                       name=f"ex_{b}_{h}_{qb}_{kcg}")
                    nc.scalar.activation(ex[:], ps[:], Exp,
                                         scale=float(SCALE), bias=bias_m4[:])
                    return ex

                def emit_av(kcg, ex):
                    for kc2 in range(2):
                        kc = kcg * 2 + kc2
                        tcg = b * NKC + kc
                        nc.tensor.matmul(
                            po[0:65, :],
                            v_nat[:, tcg, h * 65:(h + 1) * 65],
                            ex[:, kc2 * 512:(kc2 + 1) * 512],
                            start=(kc == 0), stop=(kc == NKC - 1))

                ex_prev = emit_scores(0)
                feed.tick()
                for kcg in range(1, NKC // 2):
                    ex_cur = emit_scores(kcg)
                    feed.tick()
                    emit_av(kcg - 1, ex_prev)
                    ex_prev = ex_cur
                emit_av(NKC // 2 - 1, ex_prev)
                # normalization: reciprocal on the [1,512] denominator row,
                # broadcast across 64 partitions on the idle GPSIMD engine
                s32 = small_pool.tile([1, 512], f32, tag="r",
                                      name=f"s32_{b}_{h}_{qb}")
                nc.vector.tensor_copy(s32[:], po[64:65, :])
                r32 = small_pool.tile([1, 512], f32, tag="r",
                                      name=f"r32_{b}_{h}_{qb}")
                nc.vector.reciprocal_approx_fast(r32[:], s32[:])
                rb = small_pool.tile([64, 512], f32, tag="rb",
                                     name=f"rb_{b}_{h}_{qb}")
                nc.gpsimd.partition_broadcast(rb[:], r32[:], channels=64)
                nc.vector.tensor_tensor(
                    ohT[hsl, qsl], po[0:64, :], rb[:], Mult)

            # ---- AllToAll of head h's oh rows for batch b: posted as
            # soon as that head's 4 query blocks are done ----
            def emit_a2a_stage(b, h):
                hsl = slice(h * 64, (h + 1) * 64)
                for j in range(NCORES):
                    csl = slice(b * N + j * TPC, b * N + (j + 1) * TPC)
                    nc.sync.dma_start(out=a2a_in[b][h][j], in_=ohT[hsl, csl])
                nc.gpsimd.collective_compute(
                    "AllToAll", mybir.AluOpType.bypass, replica_groups=GROUPS,
                    ins=[a2a_in[b][h][:].opt()], outs=[a2a_out[b][h][:].opt()])

            def emit_gather(b):
                # fuse the two head-halves: partitions 0:64 <- h0 rows,
                # 64:128 <- h1 rows, so the projection contracts over the
                # full 128 rows of each peer's feature chunk
                g = g_pool.tile([128, NCORES, TPC], f16, tag="g",
                                name=f"g_{b}")
                for h in range(HPC):
                    for j in range(NCORES):
                        nc.sync.dma_start(out=g[h * 64:(h + 1) * 64, j, :],
                                          in_=a2a_out[b][h][j])
                return g

            # ================= program =================
            import os as _os
            no_ilv = bool(int(_os.environ.get("KV4_NO_INTERLEAVE", "0")))

            # qkv(tb0) upfront; the rest of batch 0's qkv is slot-
            # scheduled into the attention stream (k/v of block tb due just
            # before the score/av slots that consume them), so the exp
            # pipeline starts as soon as x block 0 has landed instead of
            # after the whole x stream
            for f in qk_unit_micro(tb0 := 0, 1) + qk_unit_micro(0, 0) + \
                    v_unit_micro(0):
                f()
            sched = []
            for tb in range(1, NTB // 2):
                due = 2 * tb - 1
                sched += [(due, f) for f in qk_unit_micro(tb, 1)]
                sched += [(due, f) for f in v_unit_micro(tb)]
                sched += [(due + 4, f) for f in qk_unit_micro(tb, 0)]
            qkv1 = []
            for tb in range(NTB // 2, NTB):
                qkv1 += qk_unit_micro(tb, 0) + qk_unit_micro(tb, 1) + \
                    v_unit_micro(tb)
            nslots = NQB * (NKC // 2) * HPC
            sched += [(12 + (i * (nslots - 14)) // max(1, len(qkv1)), f)
                      for i, f in enumerate(qkv1)]
            sched.sort(key=lambda df: df[0])
            if no_ilv:
                for _, f in sched:
                    f()
                sched = []
            feed = Feed(sched, span=nslots)
            for h in range(HPC):
                for qb in range(NQB):
                    emit_attn_group(0, h, qb, feed)
                emit_a2a_stage(0, h)
            feed.flush()
            g0 = emit_gather(0)

            # attention batch 1 with batch-0 projection closures drip-fed
            # from slot 16 on (by then the batch-0 AllToAll has landed)
            proj0 = []
            for tc2 in range(2):
                for jh in range(2):
                    proj0 += proj_unit_micro(0, g0, tc2, jh)
            tail_fill = []
            if no_ilv:
                tail_fill, proj0 = proj0, []
            feed = Feed([(20 + i, f) for i, f in enumerate(proj0)],
                        span=NQB * (NKC // 2) * HPC)
            for h in range(HPC):
                for qb in range(NQB):
                    emit_attn_group(1, h, qb, feed)
                emit_a2a_stage(1, h)
            feed.flush()
            for f in tail_fill:
                f()

            g1 = emit_gather(1)
            for tc2 in range(2):
                for jh in range(2):
                    for f in proj_unit_micro(1, g1, tc2, jh):
                        f()

            if dbg:
                nc.sync.dma_start(out=dbg_d["qT"][:], in_=qT[:])
                nc.sync.dma_start(out=dbg_d["kT"][:], in_=kT[:])
                nc.sync.dma_start(
                    out=dbg_d["v"][:],
                    in_=v_nat[:].rearrange("p a b -> p (a b)"))
                nc.sync.dma_start(out=dbg_d["ohT"][:], in_=ohT[:])

    nc.compile()
    return nc


def _weights_key(w_qkv, b_qkv, w_proj):
    import hashlib
    h = hashlib.sha1()
    for a in (w_qkv, b_qkv, w_proj):
        h.update(np.ascontiguousarray(a, dtype=np.float32).tobytes())
    return h.hexdigest()


def get_program(w_qkv, b_qkv, w_proj):
    key = _weights_key(w_qkv, b_qkv, w_proj)
    if _CACHE.get("key") != key:
        _CACHE["nc"] = _build_program()
        _CACHE["weights"] = _prep_weights(w_qkv, b_qkv, w_proj)
        _CACHE["key"] = key
    return _CACHE["nc"]


def build_null_program():
    """Tiny kernel for calibrating per-dispatch overhead in test harnesses."""
    import concourse.mybir as mybir
    import concourse.tile as tile
    from concourse import bacc

    f32 = mybir.dt.float32
    nc = bacc.Bacc("TRN2", target_bir_lowering=False, debug=False,
                   num_devices=NCORES)
    x_in = nc.dram_tensor("x", [128, 128], f32, kind="ExternalInput")
    y_out = nc.dram_tensor("y", [128, 128], f32, kind="ExternalOutput")
    with tile.TileContext(nc) as tc:
        with tc.tile_pool(name="p", bufs=1) as pool:
            t = pool.tile([128, 128], f32)
            nc.sync.dma_start(out=t[:], in_=x_in[:])
            nc.sync.dma_start(out=y_out[:], in_=t[:])
    nc.compile()
    x = np.zeros((128, 128), dtype=np.float32)
    return nc, [{"x": x} for _ in range(NCORES)]


def make_in_maps(x, w_qkv=None, b_qkv=None, w_proj=None):
    """Host-side prep: full x^T fp16 in token-block-major layout (identical
    for every core) + per-core qkv weight slices (from the get_program
    cache unless weights are passed explicitly)."""
    CC = C // 128
    NTB = T // 512
    xT = np.ascontiguousarray(x.reshape(T, C).T).astype(np.float16)  # [C, T]
    # [C, T] -> [CC, 128, NTB, 512] -> [128, NTB, CC, 512]
    xs = (xT.reshape(CC, 128, NTB, 512).transpose(1, 2, 0, 3)
          .reshape(128, NTB * CC * 512))
    xs = np.ascontiguousarray(xs)
    if w_qkv is not None:
        wq_all, b_all, w2_full = _prep_weights(w_qkv, b_qkv, w_proj)
    else:
        wq_all, b_all, w2_full = _CACHE["weights"]
    return [{"xs": xs,
             "wq": np.ascontiguousarray(wq_all[c * 128:(c + 1) * 128]),
             "bq": np.ascontiguousarray(b_all[c * 128:(c + 1) * 128]),
             "w2": w2_full}
            for c in range(NCORES)]


def combine_results(results, b_qkv, w_proj, b_proj):
    """Host-side unshard: each core owns tokens [256c, 256c+256) of each
    batch; add the effective bias (v bias passes through softmax + proj)."""
    b_eff = (b_proj.astype(np.float64)
             + b_qkv[2 * C:].astype(np.float64) @ w_proj.astype(np.float64))
    acc = np.empty((B, N, C), np.float32)
    for c, res in enumerate(results):
        sh = np.asarray(res["out_sh"]).astype(np.float32)
        for b in range(B):
            acc[b, c * TPC:(c + 1) * TPC] = sh[b]
    return acc + b_eff.astype(np.float32)[None, None, :]


def kernel(x, w_qkv, b_qkv, w_proj, b_proj):
    from concourse.bass_utils import run_bass_kernel_spmd

    x = np.asarray(x, dtype=np.float32)
    w_qkv = np.asarray(w_qkv, dtype=np.float32)
    b_qkv = np.asarray(b_qkv, dtype=np.float32)
    w_proj = np.asarray(w_proj, dtype=np.float32)
    b_proj = np.asarray(b_proj, dtype=np.float32)

    nc = get_program(w_qkv, b_qkv, w_proj)
    in_maps = make_in_maps(x)
    res = run_bass_kernel_spmd(nc, in_maps, list(range(NCORES)))
    return combine_results(res.results, b_qkv, w_proj, b_proj)


# revision 39
# speedup vs baseline: 15.0427x; 15.0427x over previous
"""Multi-head self-attention Trainium2 kernel (8-core head-parallel, v3).

Problem: B=2, N=2048, C=1024, H=16 heads, HD=64.

The graded wall-time is dominated by host<->device I/O shipping (the axon
tunnel moves every NEFF ExternalInput/Output on each call at ~0.5 ms/MB/
core), so this version minimizes per-call tunnel bytes:

  * weights are baked into the NEFF as inline consts (loaded to device
    DRAM once at model-load time, not per call); each core picks its
    per-head slice with partition_id-predicated DMAs.
  * input: each core receives only ITS token shard of x (x^T slice
    [C, 512] in fp16, 1 MB); the full x^T is reassembled on-device with
    an AllGather over NeuronLink.
  * output: the 8 partial output projections are summed on-device with
    per-batch ReduceScatter(add) (batch 0's reduce overlaps batch 1's
    compute), so each core ships back only 512 rows of the final
    [4096, 1024] output in fp16 (1 MB).

Compute (per core, 2 heads): all matmuls in fp16 (double PE rate):
  1. qkv: q^T,k^T = w^T @ x^T per 512-token block (contraction over C in
     8 chunks), q/k bias added during PSUM->SBUF evacuation.  v is
     produced directly in natural [token, feat] layout (x-chunk
     stationary, w_v moving) so no PE transpose is needed; a constant
     1.0 column per head is memset so attn@v also yields the softmax
     denominators.  The v bias is folded into the host-side output bias
     (softmax rows sum to 1, so it passes through as b_v @ w_proj).
  2. attention per (batch, head): score chunks on PE, exp(s/8 - 4) on
     ACT straight out of PSUM (the -4 bias cancels in normalization and
     keeps fp16 exp comfortably in range), attn@v accumulated over 16
     key chunks into PSUM [65, 512] (row 64 = denominators).
  3. normalization: denominators broadcast across partitions via a PE
     outer product, reciprocal + multiply on DVE -> oh^T fp16.
  4. partial projection oh^T @ w2 -> DRAM bounce, then ReduceScatter.
"""

import numpy as np

B, N, C = 2, 2048, 1024
H = 16
HD = C // H  # 64
SCALE = HD ** -0.5
T = B * N  # 4096 tokens
NCORES = 8
HPC = H // NCORES  # 2 heads per core
SHARD = T // NCORES  # 512 tokens per core
OSH = N // NCORES  # 256 output rows per core per batch
EXP_BIAS = -4.0

_CACHE = {}


def _prep_weights(w_qkv, b_qkv, w_proj):
    """Stack per-core weight slices for the inline-const tensors.

    wq is stored partition-major ([128, CC*384] per core) so the SBUF load
    is one fully-contiguous DMA instead of 1024 x 768B strided rows."""
    CC = C // 128
    F = 3 * HPC * HD
    wq_all = np.empty((NCORES * 128, CC * F), np.float16)
    b_all = np.empty((NCORES * 128, 2), np.float32)
    w2_all = np.empty((NCORES * HPC * HD, C), np.float16)
    for core in range(NCORES):
        heads = [core * HPC + h for h in range(HPC)]
        cols = []
        for s in range(3):  # q, k, v groups -> [qA qB kA kB vA vB]
            for h in heads:
                cols.append(np.arange(s * C + h * HD, s * C + (h + 1) * HD))
        cols = np.concatenate(cols)
        wq_core = w_qkv[:, cols].astype(np.float16)  # [C, F]
        # [C, F] -> [p, cc, F] -> [128, CC*F]
        wq_all[core * 128:(core + 1) * 128] = (
            wq_core.reshape(CC, 128, F).transpose(1, 0, 2).reshape(128, CC * F))
        b_all[core * 128:(core + 1) * 128] = (
            b_qkv[cols[:256]].reshape(2, HPC * HD).T.astype(np.float32))
        rows = np.concatenate(
            [np.arange(h * HD, (h + 1) * HD) for h in heads])
        w2_all[core * 128:(core + 1) * 128] = w_proj[rows, :].astype(np.float16)
    return wq_all, b_all, w2_all


def _build_program(w_qkv, b_qkv, w_proj, reps=1, sim_mode=False,
                   ag_split=True, rs_split=True):
    # sim_mode: skip collectives (unsupported by TimelineSim) so the compute
    # portion can be timeline-profiled single-core; numerics are garbage.
    # ag_split/rs_split: emit the x AllGather / output ReduceScatter as two
    # halves (overlap) or one collective each (less per-collective overhead).
    import concourse.bass as bass
    import concourse.mybir as mybir
    import concourse.tile as tile
    from concourse import bacc

    f16 = mybir.dt.float16
    f32 = mybir.dt.float32
    Exp = mybir.ActivationFunctionType.Exp
    Mult = mybir.AluOpType.mult

    wq_all, b_all, w2_all = _prep_weights(w_qkv, b_qkv, w_proj)

    nc = bacc.Bacc("TRN2", target_bir_lowering=False, debug=False,
                   num_devices=NCORES)

    # x shard ships partition-major per channel-half ([2*128, 4*512]) so the
    # bounce, the gathered-block reads, and the SBUF tile are all contiguous
    CC = C // 128
    HCC = CC // 2
    xs_d = nc.dram_tensor("xs", [2 * 128, HCC * SHARD], f16,
                          kind="ExternalInput")
    out_d = nc.dram_tensor("out_sh", [B, OSH, C], f16, kind="ExternalOutput")

    wq_c = nc.inline_tensor(wq_all, "wq_c")
    b_c = nc.inline_tensor(b_all, "b_c")
    w2_c = nc.inline_tensor(w2_all, "w2_c")

    # collective bounce buffers (outputs Shared for the fast HBM-HBM path).
    # The x AllGather is split by channel halves so the qkv contraction can
    # start on ci 0..3 while the second half is still gathering.
    NAG = 2 if ag_split else 1
    HROWS = 256 // NAG  # rows per gather chunk (128 per half, or all 256)
    xg_in_h = [nc.dram_tensor(f"xg_in{h}", [HROWS, HCC * SHARD], f16,
                              kind="Internal") for h in range(NAG)]
    xg_h = [nc.dram_tensor(f"xg{h}", [NCORES * HROWS, HCC * SHARD], f16,
                           kind="Internal", addr_space="Shared")
            for h in range(NAG)]
    NRS = B if rs_split else 1
    RSROWS = T // NRS
    op_b = [nc.dram_tensor(f"op{b}", [RSROWS, C], f16, kind="Internal")
            for b in range(NRS)]
    os_b = [nc.dram_tensor(f"os{b}", [RSROWS // NCORES, C], f16,
                           kind="Internal") for b in range(NRS)]

    CC = C // 128          # 8 contraction chunks
    NTB = T // 512         # 8 token blocks (= shards)
    NKC = N // 128         # 16 key chunks per batch
    NQB = N // 512         # 4 query blocks per batch
    NTC = T // 128         # 32 token chunks
    GROUPS = [list(range(NCORES))]

    pid = nc.partition_id()

    with tile.TileContext(nc) as tc:
        with tc.tile_pool(name="persist", bufs=1) as persist, \
             tc.tile_pool(name="xt", bufs=3, space="SBUF") as xt_pool, \
             tc.tile_pool(name="exp", bufs=6) as exp_pool, \
             tc.tile_pool(name="small", bufs=4) as small_pool, \
             tc.tile_pool(name="ob", bufs=3) as out_pool, \
             tc.tile_pool(name="ps", bufs=2, space="PSUM") as psum_s, \
             tc.tile_pool(name="aux", bufs=1, space="PSUM") as psum_aux, \
             tc.tile_pool(name="po", bufs=2, space="PSUM") as psum_o:

            w_sb = persist.tile([128, CC, 3 * HPC * HD], f16, tag="w_sb")
            b_sb = persist.tile([128, 2], f32, tag="b_sb")
            w2_sb = persist.tile([128, C], f16, tag="w2_sb")
            qT = persist.tile([128, T], f16, tag="qT")
            kT = persist.tile([128, T], f16, tag="kT")
            # natural-layout v, per token-chunk: [vA(64) | 1 | vB(64) | 1]
            v_nat = persist.tile([128, NTC, 130], f16, tag="v_nat")
            ohT = persist.tile([128, T], f16, tag="ohT")
            ones64 = persist.tile([1, 64], f16, tag="ones64")
            bias_m4 = persist.tile([128, 1], f32, tag="bias_m4")

            # per-core weight selection: 8 predicated DMAs, 7 skip
            # (wq const is partition-major: contiguous [128, CC*384] rows)
            for c in range(NCORES):
                cond = pid == c
                nc.sync.dma_start(
                    out=w_sb[:],
                    in_=wq_c[c * 128:(c + 1) * 128, :].rearrange(
                        "p (cc f) -> p cc f", cc=CC),
                    cond=cond)
                nc.sync.dma_start(
                    out=b_sb[:], in_=b_c[c * 128:(c + 1) * 128, :], cond=cond)
                nc.sync.dma_start(
                    out=w2_sb[:], in_=w2_c[c * 128:(c + 1) * 128, :], cond=cond)
            nc.vector.memset(ones64[:], 1.0)
            nc.vector.memset(bias_m4[:], EXP_BIAS)

            def emit_body(rep):
                # constant 1.0 columns (per-head softmax-denominator rows)
                nc.vector.memset(v_nat[:, :, 64:65], 1.0)
                nc.vector.memset(v_nat[:, :, 129:130], 1.0)

                for h in range(NAG):
                    nc.scalar.dma_start(
                        out=xg_in_h[h][:],
                        in_=xs_d[h * HROWS:(h + 1) * HROWS, :])
                for h in range(NAG):
                    if not sim_mode:
                        nc.gpsimd.collective_compute(
                            "AllGather", mybir.AluOpType.bypass,
                            replica_groups=GROUPS,
                            ins=[xg_in_h[h][:].opt()],
                            outs=[xg_h[h][:].opt()])

                # ---- phase 1 (per batch): q^T,k^T = w^T @ x^T with bias on
                # evac; v computed in natural [token, feat] layout
                def emit_qkv(tb):
                    xt = xt_pool.tile([128, CC, 512], f16, tag="xt",
                                      name=f"xt_{rep}_{tb}")
                    for h in range(2):
                        g = h if ag_split else 0
                        r0 = tb * HROWS + (h * 128 if not ag_split else 0)
                        nc.sync.dma_start(
                            out=xt[:, h * HCC:(h + 1) * HCC, :],
                            in_=xg_h[g][r0:r0 + 128, :].rearrange(
                                "p (cc t) -> p cc t", cc=HCC))
                    xts = [xt[:, ci, :] for ci in range(CC)]
                    for fc in range(2):
                        ps = psum_s.tile([128, 512], f32, tag="s",
                                         name=f"ps1_{rep}_{tb}_{fc}")
                        for ci in range(CC):
                            nc.tensor.matmul(
                                ps[:],
                                w_sb[:, ci, fc * 128:(fc + 1) * 128],
                                xts[ci],
                                start=(ci == 0), stop=(ci == CC - 1))
                        nc.vector.tensor_scalar_add(
                            (qT if fc == 0 else kT)[:, tb * 512:(tb + 1) * 512],
                            ps[:], b_sb[:, fc:fc + 1])
                    for tcq in range(4):
                        tcg = tb * 4 + tcq
                        pv = psum_o.tile([128, 512], f32, tag="po",
                                         name=f"pv_{rep}_{tcg}")
                        for ci in range(CC):
                            nc.tensor.matmul(
                                pv[:, 0:128],
                                xt[:, ci, tcq * 128:(tcq + 1) * 128],
                                w_sb[:, ci, 256:384],
                                start=(ci == 0), stop=(ci == CC - 1))
                        # strided copy: pv cols [0:64],[64:128] land at
                        # v_nat[:, tcg, 0:64] and [65:129] (skip ones col)
                        src = pv[:, 0:128]
                        dst = v_nat[:, tcg, 0:129]
                        nc.vector.tensor_copy(
                            bass.AP(tensor=dst.tensor, offset=dst.offset,
                                    ap=[list(dst.ap[0]), [65, 2], [1, 64]]),
                            bass.AP(tensor=src.tensor, offset=src.offset,
                                    ap=[list(src.ap[0]), [64, 2], [1, 64]]))

                # ---- phase 2: attention per (batch, head) ----
                def emit_attention(b):
                    for qb in range(NQB):
                        qsl = slice(b * N + qb * 512, b * N + (qb + 1) * 512)
                        po = [psum_o.tile([128, 512], f32, tag="po",
                                          name=f"po_{rep}_{b}_{qb}_{h}")
                              for h in range(HPC)]
                        for kcg in range(NKC // 2):
                            exs = {}
                            for h in range(HPC):
                                hsl = slice(h * 64, (h + 1) * 64)
                                ps = psum_s.tile(
                                    [128, 1024], f32, tag="s",
                                    name=f"ps2_{rep}_{b}_{qb}_{kcg}_{h}")
                                for kc2 in range(2):
                                    kc = kcg * 2 + kc2
                                    ksl = slice(b * N + kc * 128,
                                                b * N + (kc + 1) * 128)
                                    nc.tensor.matmul(
                                        ps[:, kc2 * 512:(kc2 + 1) * 512],
                                        kT[hsl, ksl], qT[hsl, qsl],
                                        start=True, stop=True)
                                ex = exp_pool.tile(
                                    [128, 1024], f16, tag="ex",
                                    name=f"ex_{rep}_{b}_{qb}_{kcg}_{h}")
                                nc.scalar.activation(ex[:], ps[:], Exp,
                                                     scale=float(SCALE),
                                                     bias=bias_m4[:])
                                exs[h] = ex
                            for kc2 in range(2):
                                kc = kcg * 2 + kc2
                                tcg = b * NKC + kc
                                for h in range(HPC):
                                    nc.tensor.matmul(
                                        po[h][0:65, :],
                                        v_nat[:, tcg, h * 65:(h + 1) * 65],
                                        exs[h][:, kc2 * 512:(kc2 + 1) * 512],
                                        start=(kc == 0),
                                        stop=(kc == NKC - 1))
                        for h in range(HPC):
                            # broadcast denom row across partitions via a PE
                            # outer product, then reciprocal + multiply on DVE
                            s_sb = small_pool.tile(
                                [1, 512], f16, tag="r",
                                name=f"s_sb_{rep}_{b}_{qb}_{h}")
                            nc.vector.tensor_copy(s_sb[:], po[h][64:65, :])
                            pr = psum_aux.tile([64, 512], f32, tag="aux",
                                               name=f"pr_{rep}_{b}_{qb}_{h}")
                            nc.tensor.matmul(pr[:], ones64[:], s_sb[:],
                                             start=True, stop=True)
                            rcp = small_pool.tile(
                                [64, 512], f32, tag="rb",
                                name=f"rcp_{rep}_{b}_{qb}_{h}")
                            nc.vector.reciprocal_approx_fast(rcp[:], pr[:])
                            nc.vector.tensor_tensor(
                                ohT[h * 64:(h + 1) * 64, qsl],
                                po[h][0:64, :], rcp[:], Mult)

                        # ---- phase 3 interleaved: project this q-block's
                        # 4 token chunks while the next q-block computes ----
                        for tcq in range(4):
                            tcg = b * 16 + qb * 4 + tcq
                            pp = psum_aux.tile([128, 1024], f32, tag="aux",
                                               name=f"pp_{rep}_{tcg}")
                            for jh in range(C // 512):
                                nc.tensor.matmul(
                                    pp[:, jh * 512:(jh + 1) * 512],
                                    ohT[:, tcg * 128:(tcg + 1) * 128],
                                    w2_sb[:, jh * 512:(jh + 1) * 512],
                                    start=True, stop=True)
                            ob = out_pool.tile([128, 1024], f16, tag="ob",
                                               name=f"ob_{rep}_{tcg}")
                            nc.vector.tensor_copy(ob[:], pp[:])
                            rsb = b if rs_split else 0
                            lr = tcg * 128 - rsb * N  # row within op_b[rsb]
                            nc.sync.dma_start(
                                out=op_b[rsb][lr:lr + 128, :], in_=ob[:])

                for b in range(B):
                    for tb in range(b * NTB // B, (b + 1) * NTB // B):
                        emit_qkv(tb)
                    emit_attention(b)
                    # batch b's reduce-scatter overlaps batch b+1's compute
                    if rs_split:
                        if not sim_mode:
                            nc.gpsimd.collective_compute(
                                "ReduceScatter", mybir.AluOpType.add,
                                replica_groups=GROUPS,
                                ins=[op_b[b][:].opt()],
                                outs=[os_b[b][:].opt()])
                        nc.sync.dma_start(out=out_d[b, :, :], in_=os_b[b][:])
                if not rs_split:
                    if not sim_mode:
                        nc.gpsimd.collective_compute(
                            "ReduceScatter", mybir.AluOpType.add,
                            replica_groups=GROUPS,
                            ins=[op_b[0][:].opt()], outs=[os_b[0][:].opt()])
                    nc.sync.dma_start(out=out_d[:].opt(),
                                      in_=os_b[0][:].opt())

            for rep in range(reps):
                emit_body(rep)

    nc.compile()
    return nc


def _weights_key(w_qkv, b_qkv, w_proj):
    import hashlib
    h = hashlib.sha1()
    for a in (w_qkv, b_qkv, w_proj):
        h.update(np.ascontiguousarray(a, dtype=np.float32).tobytes())
    return h.hexdigest()


def get_program(w_qkv, b_qkv, w_proj):
    key = _weights_key(w_qkv, b_qkv, w_proj)
    if _CACHE.get("key") != key:
        _CACHE["nc"] = _build_program(w_qkv, b_qkv, w_proj)
        _CACHE["key"] = key
    return _CACHE["nc"]


def build_null_program():
    """Tiny kernel for calibrating per-dispatch overhead in test harnesses."""
    import concourse.mybir as mybir
    import concourse.tile as tile
    from concourse import bacc

    f32 = mybir.dt.float32
    nc = bacc.Bacc("TRN2", target_bir_lowering=False, debug=False,
                   num_devices=NCORES)
    x_in = nc.dram_tensor("x", [128, 128], f32, kind="ExternalInput")
    y_out = nc.dram_tensor("y", [128, 128], f32, kind="ExternalOutput")
    with tile.TileContext(nc) as tc:
        with tc.tile_pool(name="p", bufs=1) as pool:
            t = pool.tile([128, 128], f32)
            nc.sync.dma_start(out=t[:], in_=x_in[:])
            nc.sync.dma_start(out=y_out[:], in_=t[:])
    nc.compile()
    x = np.zeros((128, 128), dtype=np.float32)
    return nc, [{"x": x} for _ in range(NCORES)]


def make_in_maps(x, w_qkv=None, b_qkv=None, w_proj=None):
    """Host-side sharding: per-core input dicts (fp16 x-shard only),
    partition-major per channel-half to match the device layout."""
    HCC = C // 256
    xT = np.ascontiguousarray(x.reshape(T, C).T).astype(np.float16)
    maps = []
    for core in range(NCORES):
        xs = xT[:, core * SHARD:(core + 1) * SHARD]
        xs = xs.reshape(2, HCC, 128, SHARD).transpose(0, 2, 1, 3)
        maps.append({"xs": np.ascontiguousarray(
            xs.reshape(256, HCC * SHARD))})
    return maps


def combine_results(results, b_qkv, w_proj, b_proj):
    """Host-side unshard: interleave the per-batch output shards, add the
    effective bias (v bias passes through softmax + projection)."""
    b_eff = (b_proj.astype(np.float64)
             + b_qkv[2 * C:].astype(np.float64) @ w_proj.astype(np.float64))
    acc = np.empty((B, N, C), np.float32)
    for c, res in enumerate(results):
        sh = np.asarray(res["out_sh"]).astype(np.float32)
        for b in range(B):
            acc[b, c * OSH:(c + 1) * OSH] = sh[b]
    return acc + b_eff.astype(np.float32)[None, None, :]


def kernel(x, w_qkv, b_qkv, w_proj, b_proj):
    from concourse.bass_utils import run_bass_kernel_spmd

    x = np.asarray(x, dtype=np.float32)
    w_qkv = np.asarray(w_qkv, dtype=np.float32)
    b_qkv = np.asarray(b_qkv, dtype=np.float32)
    w_proj = np.asarray(w_proj, dtype=np.float32)
    b_proj = np.asarray(b_proj, dtype=np.float32)

    nc = get_program(w_qkv, b_qkv, w_proj)
    in_maps = make_in_maps(x)
    res = run_bass_kernel_spmd(nc, in_maps, list(range(NCORES)))
    return combine_results(res.results, b_qkv, w_proj, b_proj)

